# revision 1
# baseline (speedup 1.0000x reference)
"""Trainium2 Bass kernel for nn_DecSwitchedDeconv (switched per-sample double deconv).

Strategy (data-parallel over 8 cores, 32 samples/core, processed in pairs):
  - x padded to 34x34 in SBUF; stride-1 ConvTranspose == 3x3 conv with flipped
    kernel, realized as shift-offset bf16 matmuls accumulating f32 in PSUM
    (3 chunks of N=362, each within one PSUM bank).
  - Two samples per matmul via block-diagonal stationary weights
    (conv1 lhsT [128 = 2*64 cin, 64 = 2*32 cout]); conv2 additionally stacks
    dy=0/1 taps into K=128 using a row-shifted duplicate of h (6 matmuls per
    chunk instead of 9), with dy=2 folded into the upper partition half.
  - Per-sample branch weights gathered on-device with register-dynamic DMAs
    (y_index -> SP/Pool registers -> bass.ds row offsets) from pre-zero-padded
    A/B-position DRAM tables, so gathered pair tiles are block-diagonal with
    one contiguous descriptor per row.
  - bias+relu on ScalarE from PSUM; epilogue fused as (psum + b2) * z on
    VectorE, then residual add; I/O DMAs on sync (HW DGE), conv2 weight
    gathers on the gpsimd queue.
"""

import numpy as np

import concourse.bacc as bacc
import concourse.bass as bass
import concourse.mybir as mybir
import concourse.tile as tile
from concourse.bass_utils import run_bass_kernel_spmd

B, C, CSM, NB, HW = 256, 64, 32, 8, 32
M = 8                  # cores
BS = B // M            # 32 samples per core
NPAIR = BS // 2        # 16
WP = HW + 2            # 34 padded width
L = WP * WP            # 1156
NVAL = (HW - 1) * WP + HW   # 1086: contiguous span covering all valid outputs
BASE = WP + 1          # 35: flat offset of (y=1,x=1)
NCH = 3
CH = NVAL // NCH       # 362 matmul chunk (>=256 keeps float32r at 1 cyc/row)
NBUF = 4               # ping-pong depth for persistent per-pair buffers

f32 = mybir.dt.float32
bf16 = mybir.dt.bfloat16
i32 = mybir.dt.int32


def _build_bass():
    nc = bacc.Bacc(target_bir_lowering=False, debug=False)
    xs = nc.dram_tensor("xs", [BS * C, HW * HW], f32, kind="ExternalInput")
    y32 = nc.dram_tensor("y32", [BS, 1], i32, kind="ExternalInput")
    zs = nc.dram_tensor("zs", [BS * C, 1], f32, kind="ExternalInput")
    w1ga = nc.dram_tensor("w1ga", [NB * C, 9 * 2 * CSM], bf16, kind="ExternalInput")
    w1gb = nc.dram_tensor("w1gb", [NB * C, 9 * 2 * CSM], bf16, kind="ExternalInput")
    w2gla = nc.dram_tensor("w2gla", [NB * CSM, 6 * 2 * C], bf16, kind="ExternalInput")
    w2glb = nc.dram_tensor("w2glb", [NB * CSM, 6 * 2 * C], bf16, kind="ExternalInput")
    w2gua = nc.dram_tensor("w2gua", [NB * CSM, 6 * 2 * C], bf16, kind="ExternalInput")
    w2gub = nc.dram_tensor("w2gub", [NB * CSM, 6 * 2 * C], bf16, kind="ExternalInput")
    b1g = nc.dram_tensor("b1g", [NB * CSM, 1], f32, kind="ExternalInput")
    b2g = nc.dram_tensor("b2g", [NB * C, 1], f32, kind="ExternalInput")
    outd = nc.dram_tensor("out", [BS * C, HW * HW], f32, kind="ExternalOutput")

    mul = mybir.AluOpType.mult
    add = mybir.AluOpType.add

    with tile.TileContext(nc) as tc:
        # y_index as an i32 row on partition 0 — register gathers read it
        # directly (engine register loads only need partition 0)
        ybc = nc.alloc_sbuf_tensor("ybc", [1, BS], i32).ap()
        nc.sync.dma_start(ybc, bass.AP(y32.ap().tensor, 0, [[BS, 1], [1, BS]]))

        # ---- persistent ping-pong buffers (zeroed once; borders/off-blocks
        # stay zero because per-pair writes never touch them) ----
        xpads, hpads, wt1s, wt2s = [], [], [], []
        for i in range(NBUF):
            xpads.append(nc.alloc_sbuf_tensor(f"xpad{i}", [128, L], bf16).ap())
            hpads.append(nc.alloc_sbuf_tensor(f"hpad{i}", [128, L], bf16).ap())
            wt1s.append(nc.alloc_sbuf_tensor(f"wt1{i}", [128, 9 * 2 * CSM], bf16).ap())
            wt2s.append(nc.alloc_sbuf_tensor(f"wt2{i}", [128, 6 * 2 * C], bf16).ap())
            nc.vector.memset(xpads[i], 0.0)
            # hpad only needs its two contiguous border strips zeroed: the
            # relu1 span + colfix + shift-copy rewrite everything else used
            nc.vector.memset(hpads[i][0:64, 0:BASE], 0.0)
            nc.vector.memset(hpads[i][0:64, BASE + NVAL:L], 0.0)

        with (
            tc.tile_pool(name="io", bufs=3) as iop,
            tc.tile_pool(name="sml", bufs=3) as smlp,
            tc.tile_pool(name="ps1", bufs=3, space="PSUM") as ps1p,
            tc.tile_pool(name="ps2", bufs=3, space="PSUM") as ps2p,
        ):
            for p in range(NPAIR):
                bi = p % NBUF
                xpad, hpad, wt1, wt2 = xpads[bi], hpads[bi], wt1s[bi], wt2s[bi]

                xraw = iop.tile([128, HW * HW], f32, tag="xraw")
                opad = iop.tile([128, L], f32, tag="opad")
                ot = iop.tile([128, HW * HW], f32, tag="ot")
                ztile = smlp.tile([128, 1], f32, tag="z")
                b1t = smlp.tile([64, 1], f32, tag="b1")
                b2t = smlp.tile([128, 1], f32, tag="b2")

                # input loads
                nc.sync.dma_start(xraw[:, :], xs.ap()[2 * p * 64:(2 * p + 2) * 64, :])
                nc.sync.dma_start(ztile[:, :], zs.ap()[2 * p * 64:(2 * p + 2) * 64, :])

                # per-sample weight/bias gathers: register-dynamic DMA of
                # full pre-zero-padded block-diagonal rows (w1ga has sample-A
                # column positions populated, w1gb sample-B; so the gathered
                # pair tile is block-diagonal with zero blocks straight from
                # DRAM, one contiguous descriptor per partition row).
                for s in range(2):
                    r = nc.alloc_register(mybir.EngineType.SP, f"gy{p}_{s}")
                    nc.sync.load(r, ybc[0:1, 2 * p + s:2 * p + s + 1])
                    nc.sync.reg_mul(r, r, 64)
                    v64 = nc.snap(r, donate=True, min_val=0, max_val=448)
                    nc.sync.dma_start(
                        wt1[64 * s:64 * (s + 1), :],
                        (w1ga if s == 0 else w1gb).ap()[bass.ds(v64, 64), :])
                    # conv2 stacked-weight + bias gathers on the gpsimd queue
                    rp = nc.alloc_register(mybir.EngineType.Pool, f"py{p}_{s}")
                    rp2 = nc.alloc_register(mybir.EngineType.Pool, f"pz{p}_{s}")
                    nc.gpsimd.load(rp, ybc[0:1, 2 * p + s:2 * p + s + 1])
                    nc.gpsimd.reg_alu(rp2, rp, 64, mul)
                    nc.gpsimd.reg_mul(rp, rp, 32)
                    vp32 = nc.snap(rp, donate=True, min_val=0, max_val=224)
                    vp64 = nc.snap(rp2, donate=True, min_val=0, max_val=448)
                    lo, up = (w2gla, w2gua) if s == 0 else (w2glb, w2gub)
                    nc.gpsimd.dma_start(
                        wt2[32 * s:32 * (s + 1), :], lo.ap()[bass.ds(vp32, 32), :])
                    nc.gpsimd.dma_start(
                        wt2[64 + 32 * s:64 + 32 * (s + 1), :],
                        up.ap()[bass.ds(vp32, 32), :])
                    nc.gpsimd.dma_start(
                        b1t[32 * s:32 * (s + 1), :], b1g.ap()[bass.ds(vp32, 32), :])
                    nc.gpsimd.dma_start(
                        b2t[64 * s:64 * (s + 1), :], b2g.ap()[bass.ds(vp64, 64), :])

                # relu(x) into padded layout (interior only; borders stay 0)
                xpad3 = xpad.rearrange("p (h w) -> p h w", w=WP)[:, 1:HW + 1, 1:HW + 1]
                xraw3 = xraw[:, :].rearrange("p (h w) -> p h w", w=HW)
                nc.scalar.activation(xpad3, xraw3, mybir.ActivationFunctionType.Relu)

                # conv1: 3 chunks x 9 taps, then bias+relu into hpad
                for c in range(NCH):
                    ps1 = ps1p.tile([64, CH], f32)
                    for t in range(9):
                        dy, dx = divmod(t, 3)
                        off = dy * WP + dx + c * CH
                        nc.tensor.matmul(
                            ps1[:, :],
                            lhsT=wt1[:, t * 2 * CSM:(t + 1) * 2 * CSM],
                            rhs=xpad[:, off:off + CH],
                            start=(t == 0), stop=(t == 8),
                        )
                    nc.scalar.activation(
                        hpad[0:64, BASE + c * CH:BASE + (c + 1) * CH], ps1[:, :],
                        mybir.ActivationFunctionType.Relu, bias=b1t[:, :],
                    )
                # re-zero the inter-row pad columns the relu span polluted
                hp3 = hpad.rearrange("p (h w) -> p h w", w=WP)
                nc.vector.memset(hp3[0:64, 1:HW + 1, 0:WP:WP - 1], 0.0)
                # duplicate h into rows 64:128 shifted left by one image row, so
                # one matmul covers taps dy=0 (rows 0:64) and dy=1 (rows 64:128)
                nc.vector.tensor_copy(hpad[64:128, 0:L - 34], hpad[0:64, 34:L])

                # conv2 + epilogue (z * (conv + b2) fused via tensor_scalar)
                for c in range(NCH):
                    ps2 = ps2p.tile([128, CH], f32)
                    for dx in range(3):
                        nc.tensor.matmul(
                            ps2[:, :],
                            lhsT=wt2[:, dx * 2 * C:(dx + 1) * 2 * C],
                            rhs=hpad[:, dx + c * CH:dx + c * CH + CH],
                            start=(dx == 0), stop=False,
                        )
                    for dx in range(3):
                        off = WP + dx + c * CH
                        nc.tensor.matmul(
                            ps2[:, :],
                            lhsT=wt2[64:128, (3 + dx) * 2 * C:(4 + dx) * 2 * C],
                            rhs=hpad[64:128, off:off + CH],
                            start=False, stop=(dx == 2),
                        )
                    nc.vector.tensor_scalar(
                        opad[:, BASE + c * CH:BASE + (c + 1) * CH], ps2[:, :],
                        b2t[:, :], ztile[:, :], op0=add, op1=mul,
                    )

                # residual add on the valid interior, store
                opad3 = opad[:, :].rearrange("p (h w) -> p h w", w=WP)[:, 1:HW + 1, 1:HW + 1]
                ot3 = ot[:, :].rearrange("p (h w) -> p h w", w=HW)
                nc.vector.tensor_tensor(ot3, opad3, xraw3, op=add)
                nc.sync.dma_start(outd.ap()[2 * p * 64:(2 * p + 2) * 64, :], ot[:, :])

    nc.compile()
    return nc


# enable walrus's redundant-LDWEIGHTS elision (off by default in bass_utils):
# with tap-outer loops, consecutive matmuls share the stationary operand.
import concourse.bass_utils as _bu
if not getattr(_bu, "_ldw_opt_patched", False):
    _orig_run_command = _bu.run_command
    def _run_command_ldw(argv, **kw):
        argv = list(argv)  # ldw-opt=true fails walrus visitInstLdweights; keep default
        return _orig_run_command(argv, **kw)
    _bu.run_command = _run_command_ldw
    _bu._ldw_opt_patched = True

_NC = None


def _get_nc():
    global _NC
    if _NC is None:
        _NC = _build_bass()
    return _NC


def _host_prep(x, y_index, z, W1, b1, W2, b2):
    # flipped-kernel, tap-major/cout-minor per-branch stacks
    import ml_dtypes
    w1t = np.ascontiguousarray(
        W1[:, :, :, ::-1, ::-1].transpose(0, 1, 3, 4, 2)
    ).reshape(NB * C, 9, CSM).astype(ml_dtypes.bfloat16)
    w2t = np.ascontiguousarray(
        W2[:, :, :, ::-1, ::-1].transpose(0, 1, 3, 4, 2)
    ).reshape(NB * CSM, 9, C).astype(ml_dtypes.bfloat16)
    w1ga = np.zeros((NB * C, 9, 2 * CSM), dtype=ml_dtypes.bfloat16)
    w1gb = np.zeros_like(w1ga)
    w1ga[:, :, :CSM] = w1t
    w1gb[:, :, CSM:] = w1t
    w1ga, w1gb = w1ga.reshape(NB * C, -1), w1gb.reshape(NB * C, -1)
    # conv2 stacked tables: [dx-block 0..2] = dy rows; [dx-block 3..5] = dy2
    # (read only from the upper partition half). L tables feed rows 0:64
    # (dy=0 taps), U tables rows 64:128 (dy=1 taps + dy=2 taps).
    w2gl = np.zeros((2, NB * CSM, 6, 2 * C), dtype=ml_dtypes.bfloat16)
    w2gu = np.zeros((2, NB * CSM, 6, 2 * C), dtype=ml_dtypes.bfloat16)
    for s in range(2):
        cs = slice(s * C, (s + 1) * C)
        for dx in range(3):
            w2gl[s][:, dx, cs] = w2t[:, dx, :]
            w2gu[s][:, dx, cs] = w2t[:, 3 + dx, :]
            w2gu[s][:, 3 + dx, cs] = w2t[:, 6 + dx, :]
    w2gla, w2glb = w2gl[0].reshape(NB * CSM, -1), w2gl[1].reshape(NB * CSM, -1)
    w2gua, w2gub = w2gu[0].reshape(NB * CSM, -1), w2gu[1].reshape(NB * CSM, -1)
    b1g = b1.reshape(NB * CSM, 1).astype(np.float32)
    b2g = b2.reshape(NB * C, 1).astype(np.float32)

    in_maps = []
    for c in range(M):
        sl = slice(c * BS, (c + 1) * BS)
        in_maps.append(dict(
            xs=np.ascontiguousarray(x[sl]).reshape(BS * C, HW * HW).astype(np.float32),
            y32=y_index[sl].reshape(BS, 1).astype(np.int32),
            zs=np.ascontiguousarray(z[sl]).reshape(BS * C, 1).astype(np.float32),
            w1ga=w1ga, w1gb=w1gb, w2gla=w2gla, w2glb=w2glb,
            w2gua=w2gua, w2gub=w2gub, b1g=b1g, b2g=b2g,
        ))
    return in_maps


def kernel(x, y_index, y_hard, z, W1, b1, W2, b2, _trace=False):
    x = np.asarray(x, dtype=np.float32)
    z = np.asarray(z, dtype=np.float32)
    y_index = np.asarray(y_index)
    W1 = np.asarray(W1, dtype=np.float32)
    b1 = np.asarray(b1, dtype=np.float32)
    W2 = np.asarray(W2, dtype=np.float32)
    b2 = np.asarray(b2, dtype=np.float32)

    nc = _get_nc()
    in_maps = _host_prep(x, y_index, z, W1, b1, W2, b2)
    res = run_bass_kernel_spmd(nc, in_maps, core_ids=list(range(M)), trace=_trace)
    out = np.concatenate(
        [r["out"].reshape(BS, C, HW, HW) for r in res.results], axis=0
    )
    if _trace:
        kernel._last_results = res
    return out

